# revision 6
# baseline (speedup 1.0000x reference)
"""Multi-head attention (length-masked) Trainium2 Bass kernel.

Full inputs -> shard 16 (b,t) pairs over 8 cores (2 each) -> SPMD Tile
kernel -> gather.

Math per (b,t) pair (S=1024, H=8, DK=32, D=256):
  Q^T = Wq^T X_q^T (scaled by 1/sqrt(DK)), K^T = Wk^T X_k^T, V = X_v Wv
  qhat_h = [Q_h^T/sqrt(dk); ones; maskbias]   (34 x S)
  khat_h = [K_h^T; maskbias; ones]            (34 x S)
  sT_h[k,q] = qk/sqrt(dk) + maskbias[k] + maskbias[q]   (via one matmul, K=34)
  pT = exp(sT)            (no max-sub needed: |scores| << 1; masked -> exp(-3e4)=0)
  o_aug_h[0:32,q] = V_h^T pT ; o_aug_h[32,q] = sum_k pT = l[q]   (vhat ones col)
  o_aug_h += colsum(vhat_h) (x) ind        ind = (1-qv)/S  (invalid-q rows -> mean(V))
  scale[q] = qv*1/l' + (1-qv)              l' = l + (1-qv)
  out^T = Wo^T ( O^T * scale_bcast )       (+ bo)
Host transposes per-pair outputs back to [B,T,S,D].
"""

import numpy as np

import concourse.bass as bass
import concourse.mybir as mybir
from concourse import bacc, tile
from concourse.bass_utils import run_bass_kernel_spmd

F32 = mybir.dt.float32
AF = mybir.ActivationFunctionType
ALU = mybir.AluOpType

B, T, S, D = 4, 4, 1024, 256
H, DK = 8, 32
NPAIR = 2          # (b,t) pairs per core
NCORES = 8
NEG = -30000.0
SCALE = 1.0 / np.sqrt(DK)

_cache = {}


def _build(with_bias: bool):
    nc = bacc.Bacc("TRN2", target_bir_lowering=False, debug=False,
                   num_devices=NCORES)

    din = {}
    for nm in ("xqT", "xkT", "xvT"):
        din[nm] = nc.dram_tensor(nm, [NPAIR, D, S], F32, kind="ExternalInput")
    for nm in ("wq", "wk", "wv", "woT"):
        din[nm] = nc.dram_tensor(nm, [D, D], F32, kind="ExternalInput")
    din["aug_q"] = nc.dram_tensor("aug_q", [NPAIR, 2, S], F32, kind="ExternalInput")
    din["aug_k"] = nc.dram_tensor("aug_k", [NPAIR, 2, S], F32, kind="ExternalInput")
    din["e2"] = nc.dram_tensor("e2", [128, 128], F32, kind="ExternalInput")
    din["ind"] = nc.dram_tensor("ind", [NPAIR, S], F32, kind="ExternalInput")
    if with_bias:
        for nm in ("bq", "bk", "bv", "bo"):
            din[nm] = nc.dram_tensor(nm, [1, D], F32, kind="ExternalInput")
    dout = nc.dram_tensor("out", [NPAIR, D, S], F32, kind="ExternalOutput")

    with tile.TileContext(nc) as tc:
        _emit(nc, tc, din, dout, with_bias)
    nc.compile()
    return nc


def _emit(nc, tc, din, dout, with_bias):
    P = 128
    with (
        tc.tile_pool(name="wpool", bufs=1) as wpool,
        tc.tile_pool(name="xpool", bufs=6) as xpool,
        tc.tile_pool(name="qkpool", bufs=2) as qkpool,
        tc.tile_pool(name="vpool", bufs=10) as vpool,
        tc.tile_pool(name="ppool", bufs=3) as ppool,
        tc.tile_pool(name="opool", bufs=2) as opool,
        tc.tile_pool(name="spool", bufs=1) as spool,
        tc.tile_pool(name="outpool", bufs=2) as outpool,
        tc.tile_pool(name="psumA", bufs=2, space=bass.MemorySpace.PSUM) as psumA,
        tc.tile_pool(name="psumB", bufs=2, space=bass.MemorySpace.PSUM) as psumB,
    ):
        # --- static weights ---
        w_sb = {}
        for nm in ("wq", "wk", "wv", "woT"):
            tiles = []
            for kt in range(2):
                wt = wpool.tile([P, D], F32, tag=f"{nm}{kt}", name=f"{nm}{kt}")
                nc.sync.dma_start(wt[:], din[nm].ap()[kt * P:(kt + 1) * P, :])
                tiles.append(wt)
            w_sb[nm] = tiles
        ones_col = wpool.tile([P, 1], F32, tag="ones_col")
        nc.gpsimd.memset(ones_col[:], 1.0)
        ones_row = wpool.tile([1, S], F32, tag="ones_row")
        nc.gpsimd.memset(ones_row[:], 1.0)
        b_sb = {}
        if with_bias:
            for nm in ("bq", "bk", "bv", "bo"):
                bt = wpool.tile([1, D], F32, tag=nm, name=nm)
                nc.sync.dma_start(bt[:], din[nm].ap()[:, :])
                b_sb[nm] = bt

        HOFF = [(h // 2, 64 * (h % 2)) for h in range(H)]  # (tile, row offset)

        # selector: scb[i, q] = rhs[32*(i//32), q]
        e2_sb = wpool.tile([P, P], F32, tag="e2", name="e2_sb")
        nc.sync.dma_start(e2_sb[:], din["e2"].ap()[:, :])

        for p in range(NPAIR):
            # ---------------- load inputs ----------------
            def load_x(nm):
                ts = []
                for kt in range(2):
                    xt = xpool.tile([P, S], F32, tag="x", name="x")
                    nc.sync.dma_start(xt[:], din[nm].ap()[p, kt * P:(kt + 1) * P, :])
                    ts.append(xt)
                return ts
            xq = load_x("xqT")
            xk = load_x("xkT")
            xv = load_x("xvT")

            # ---------------- Q-hat / K-hat projections ----------------
            def proj_qk(xin, wname, bname, hp_tag, scale, aug_name):
                hp = [qkpool.tile([P, S], F32, tag=f"{hp_tag}{g}",
                                  name=f"{hp_tag}{g}")
                      for g in range(4)]
                for dt in range(2):
                    ps = psumA.tile([P, S], F32, tag="ps", name="ps")
                    for sh in range(2):
                        sl = slice(sh * 512, (sh + 1) * 512)
                        for kt in range(2):
                            nc.tensor.matmul(
                                ps[:, sl],
                                lhsT=w_sb[wname][kt][:, dt * P:(dt + 1) * P],
                                rhs=xin[kt][:, sl],
                                start=(kt == 0),
                                stop=(kt == 1 and not with_bias),
                            )
                        if with_bias:
                            nc.tensor.matmul(
                                ps[:, sl],
                                lhsT=b_sb[bname][0:1, dt * P:(dt + 1) * P],
                                rhs=ones_row[0:1, sl],
                                start=False, stop=True,
                            )
                    for hl in range(4):
                        h = dt * 4 + hl
                        g, off = HOFF[h]
                        src = ps[hl * 32:(hl + 1) * 32, :]
                        dst = hp[g][off:off + 32, :]
                        if scale is None:
                            nc.vector.tensor_copy(dst, src)
                        else:
                            nc.vector.tensor_scalar_mul(dst, src, scale)
                # augmented rows: one [2, S] DMA per head slot
                aug = din[aug_name].ap()[p]
                for h in range(H):
                    g, off = HOFF[h]
                    nc.sync.dma_start(hp[g][off + 32:off + 34, :], aug)
                return hp

            qhp = proj_qk(xq, "wq", "bq", "qh", SCALE, aug_name="aug_q")
            khp = proj_qk(xk, "wk", "bk", "kh", None, aug_name="aug_k")

            # ---------------- V-hat projection ----------------
            vh = []
            for st in range(8):
                ps = psumA.tile([P, S], F32, tag="ps", name="ps")
                for kt in range(2):
                    nc.tensor.matmul(
                        ps[:, 0:D],
                        lhsT=xv[kt][:, st * P:(st + 1) * P],
                        rhs=w_sb["wv"][kt][:, :],
                        start=(kt == 0),
                        stop=(kt == 1 and not with_bias),
                    )
                if with_bias:
                    nc.tensor.matmul(
                        ps[:, 0:D],
                        lhsT=ones_row[0:1, 0:P],
                        rhs=b_sb["bv"][0:1, :],
                        start=False, stop=True,
                    )
                v = vpool.tile([P, H * 33], F32, tag="vhat", name="vhat")
                v3 = v[:].rearrange("p (h c) -> p h c", c=33)
                nc.gpsimd.memset(v3[:, :, 32:33], 1.0)
                nc.vector.tensor_copy(
                    v3[:, :, 0:32],
                    ps[:, 0:D].rearrange("p (h c) -> p h c", c=32),
                )
                vh.append(v)

            # column sums of V-hat (for invalid-q rows)
            msum = psumA.tile([P, S], F32, tag="ps", name="ps")
            for st in range(8):
                nc.tensor.matmul(
                    msum[0:1, 0:H * 33],
                    lhsT=ones_col[:, 0:1],
                    rhs=vh[st][:, :],
                    start=(st == 0), stop=(st == 7),
                )
            meanv_sb = spool.tile([1, H * 33], F32, tag="meanv", name="meanv")
            nc.vector.tensor_copy(meanv_sb[:], msum[0:1, 0:H * 33])

            # ---------------- per-pair vectors ----------------
            l_stage = [spool.tile([P, S], F32, tag=f"lstage{i}", name=f"lstage{i}")
                       for i in range(2)]
            for i in range(2):
                nc.gpsimd.memset(l_stage[i][:], 1.0)
            ind_sb = spool.tile([1, S], F32, tag="ind", name="ind")
            nc.sync.dma_start(ind_sb[:], din["ind"].ap()[p:p + 1, :])

            optile = [opool.tile([P, S], F32, tag=f"optile{dt}", name=f"optile{dt}")
                      for dt in range(2)]

            # ---------------- attention per head ----------------
            for h in range(H):
                g, off = HOFF[h]
                qh = qhp[g][off:off + 34, :]
                kh = khp[g][off:off + 34, :]
                o_aug = psumB.tile([33, S], F32, tag="oaug", name="oaug")
                for kt in range(8):
                    sT = psumA.tile([P, S], F32, tag="ps")
                    for q2 in range(2):
                        sl = slice(q2 * 512, (q2 + 1) * 512)
                        nc.tensor.matmul(
                            sT[:, sl],
                            lhsT=kh[:, kt * P:(kt + 1) * P],
                            rhs=qh[:, sl],
                            start=True, stop=True,
                        )
                    pT = ppool.tile([P, S], F32, tag="pT", name="pT")
                    nc.scalar.activation(pT[:], sT[:], AF.Exp)
                    for q2 in range(2):
                        sl = slice(q2 * 512, (q2 + 1) * 512)
                        nc.tensor.matmul(
                            o_aug[:, sl],
                            lhsT=vh[kt][:, h * 33:h * 33 + 33],
                            rhs=pT[:, sl],
                            start=(kt == 0), stop=False,
                        )
                # += colsum(vhat) (x) ind
                for q2 in range(2):
                    sl = slice(q2 * 512, (q2 + 1) * 512)
                    nc.tensor.matmul(
                        o_aug[:, sl],
                        lhsT=meanv_sb[0:1, h * 33:h * 33 + 33],
                        rhs=ind_sb[0:1, sl],
                        start=False, stop=True,
                    )
                dt, hl = h // 4, h % 4
                nc.vector.tensor_copy(optile[dt][hl * 32:(hl + 1) * 32, :],
                                      o_aug[0:32, :])
                nc.vector.tensor_copy(
                    l_stage[h // 4][32 * (h % 4):32 * (h % 4) + 1, :],
                    o_aug[32:33, :])

            # ---------------- normalize + combine scales ----------------
            # l' = l + (1-qv) already (term2 adds S*ind = 1-qv to the l row)
            for dt in range(2):
                nc.vector.reciprocal(l_stage[dt][:], l_stage[dt][:])
                scb = psumA.tile([P, S], F32, tag="ps", name="ps")
                for q2 in range(2):
                    sl = slice(q2 * 512, (q2 + 1) * 512)
                    nc.tensor.matmul(
                        scb[:, sl],
                        lhsT=e2_sb[:, :],
                        rhs=l_stage[dt][:, sl],
                        start=True, stop=True,
                    )
                nc.vector.scalar_tensor_tensor(
                    optile[dt][:], optile[dt][:], 1.0, scb[:],
                    op0=ALU.mult, op1=ALU.mult)

            # ---------------- output projection ----------------
            for ot in range(2):
                ps = psumA.tile([P, S], F32, tag="ps", name="ps")
                for sh in range(2):
                    sl = slice(sh * 512, (sh + 1) * 512)
                    for dt in range(2):
                        nc.tensor.matmul(
                            ps[:, sl],
                            lhsT=w_sb["woT"][dt][:, ot * P:(ot + 1) * P],
                            rhs=optile[dt][:, sl],
                            start=(dt == 0),
                            stop=(dt == 1 and not with_bias),
                        )
                    if with_bias:
                        nc.tensor.matmul(
                            ps[:, sl],
                            lhsT=b_sb["bo"][0:1, ot * P:(ot + 1) * P],
                            rhs=ones_row[0:1, sl],
                            start=False, stop=True,
                        )
                outT = outpool.tile([P, S], F32, tag="outT", name="outT")
                nc.vector.tensor_copy(outT[:], ps[:])
                nc.sync.dma_start(dout.ap()[p, ot * P:(ot + 1) * P, :], outT[:])


def _prep_inputs(inputs):
    query = np.ascontiguousarray(np.asarray(inputs["query"], np.float32))
    key = np.ascontiguousarray(np.asarray(inputs["key"], np.float32))
    value = np.ascontiguousarray(np.asarray(inputs["value"], np.float32))
    mask = np.asarray(inputs["mask"]).astype(np.int64)
    # reference projections are x @ W.T (einsum 'btsf,df->btsd'):
    # matmul lhsT needs [f, d] = W.T
    Wq = np.ascontiguousarray(np.asarray(inputs["Wq"], np.float32).T)
    Wk = np.ascontiguousarray(np.asarray(inputs["Wk"], np.float32).T)
    Wv = np.ascontiguousarray(np.asarray(inputs["Wv"], np.float32).T)
    WoT = np.ascontiguousarray(np.asarray(inputs["Wo"], np.float32).T)
    biases = {nm: np.asarray(inputs[nm], np.float32).reshape(1, D)
              for nm in ("bq", "bk", "bv", "bo")}
    with_bias = any(np.any(v) for v in biases.values())

    pairs = [(b, t) for b in range(B) for t in range(T)]
    core_pairs = [pairs[2 * c:2 * c + 2] for c in range(NCORES)]

    arange = np.arange(S)
    E2 = np.zeros((128, 128), np.float32)
    for i in range(128):
        E2[32 * (i // 32), i] = 1.0
    in_maps = []
    for c in range(NCORES):
        cp = core_pairs[c]
        xqT = np.stack([np.ascontiguousarray(query[b, t].T) for b, t in cp])
        xkT = np.stack([np.ascontiguousarray(key[b, t].T) for b, t in cp])
        xvT = np.stack([np.ascontiguousarray(value[b, t].T) for b, t in cp])
        lens = np.array([mask[b, t] for b, t in cp])
        qv = (arange[None, :] < lens[:, None]).astype(np.float32)  # [2,S]
        maskbias = (1.0 - qv) * NEG
        ones = np.ones_like(qv)
        m = {
            "xqT": xqT, "xkT": xkT, "xvT": xvT,
            "wq": Wq, "wk": Wk, "wv": Wv, "woT": WoT,
            "aug_q": np.ascontiguousarray(np.stack([ones, maskbias], axis=1)),
            "aug_k": np.ascontiguousarray(np.stack([maskbias, ones], axis=1)),
            "e2": E2,
            "ind": (1.0 - qv) / np.float32(S),
        }
        if with_bias:
            m.update(biases)
        in_maps.append(m)
    return in_maps, core_pairs, with_bias


def _assemble(results, core_pairs):
    out = np.empty((B, T, S, D), np.float32)
    for c in range(NCORES):
        o = results[c]["out"]  # [NPAIR, D, S]
        for j, (b, t) in enumerate(core_pairs[c]):
            out[b, t] = np.asarray(o[j]).T
    return out


def _get_program(with_bias):
    if with_bias not in _cache:
        _cache[with_bias] = _build(with_bias)
    return _cache[with_bias]


def kernel(**inputs):
    in_maps, core_pairs, with_bias = _prep_inputs(inputs)
    nc = _get_program(with_bias)
    res = run_bass_kernel_spmd(nc, in_maps, list(range(NCORES)))
    return _assemble(res.results, core_pairs)


def _install_ntff_shim(so_path="/opt/axon/libaxon_pjrt.so"):
    """Provide antenv.axon_hooks (missing in this image) so
    run_bass_kernel_spmd(trace=True) can capture NTFF profiles."""
    import sys
    import types
    import ctypes
    import contextlib

    try:
        from antenv.axon_hooks import get_axon_ntff_profile_hook  # noqa: F401
        return
    except ImportError:
        pass

    lib = ctypes.CDLL(so_path)
    if not hasattr(lib, "axon_start_nrt_profile"):
        return
    lib.axon_start_nrt_profile.argtypes = [ctypes.POINTER(ctypes.c_int64),
                                           ctypes.c_size_t]
    lib.axon_start_nrt_profile.restype = ctypes.c_int64
    lib.axon_stop_nrt_profile.argtypes = [ctypes.c_char_p]
    lib.axon_stop_nrt_profile.restype = ctypes.c_int64

    @contextlib.contextmanager
    def _hook(output_dir, device_ids):
        import jax
        jax.devices()
        if device_ids:
            ids = (ctypes.c_int64 * len(device_ids))(*device_ids)
            rc = lib.axon_start_nrt_profile(ids, len(device_ids))
        else:
            rc = lib.axon_start_nrt_profile(None, 0)
        if rc != 0:
            raise RuntimeError(f"axon_start_nrt_profile rc={rc}")
        try:
            yield
        finally:
            n = lib.axon_stop_nrt_profile(str(output_dir).encode())
            print(f"ntff profile: {n} file(s) written to {output_dir}")

    holder = {"h": _hook}
    mod = types.ModuleType("antenv.axon_hooks")
    mod.set_axon_ntff_profile_hook = lambda h: holder.__setitem__("h", h)
    mod.get_axon_ntff_profile_hook = lambda: holder.get("h")
    sys.modules["antenv.axon_hooks"] = mod
    import antenv
    antenv.axon_hooks = mod


def run_traced(**inputs):
    """test.py helper: returns (out, BassKernelResults with profile)."""
    _install_ntff_shim()
    in_maps, core_pairs, with_bias = _prep_inputs(inputs)
    nc = _get_program(with_bias)
    res = run_bass_kernel_spmd(nc, in_maps, list(range(NCORES)), trace=True)
    return _assemble(res.results, core_pairs), res
